# revision 1
# baseline (speedup 1.0000x reference)
"""Trainium2 Bass kernel for nn_Blur: depthwise 4x4 FIR blur (upfirdn2d pad=(2,1)).

Full inputs in, full output out. Internally shards the 4096 (b,c) images
across 8 NeuronCores (pure data parallel, no collectives).

v2 (bf16): tolerance is rel_err < 2e-2, so all device I/O is bf16 (host-side
RNE cast) — halves HBM traffic vs fp32 (memory-regime roofline win).  The
host also pre-arranges x into the exact SBUF layout the kernel wants
([h, img-major cols] with 2 zero gap columns per image), so every DMA is a
single large per-partition-contiguous transfer (~1.6 MB, 12.5 KB/partition
descriptors) instead of many 512 B-chunk strided ones.

Compute per core (512 images of [H=128, W=128]): the 4x4 depthwise conv
factors into 4 column-convolutions along H, each a banded matmul with the
contraction over the partition (H) axis, with the W-shift (j-2) realized as
a shifted moving-operand read of an accumulating matmul:
  psum[:, c] += W_j^T @ x[:, c + (j-2)]     W_j[hi, ho] = wf[hi-ho+2, j]
Images are packed at stride 130 (2 zero gap cols) so shifted reads pick up
zero padding at image edges.  Groups of 3 images share a PSUM bank; 4
groups (4 banks) form one chunk tile so PSUM->SBUF evacuation is one big
Vector/Scalar copy per chunk.
"""

import os
import sys
from contextlib import ExitStack

for _p in ("/opt/trn_rl_repo", "/root/.axon_site/_ro/trn_rl_repo"):
    if os.path.isdir(_p) and _p not in sys.path:
        sys.path.append(_p)

import ml_dtypes
import numpy as np

import concourse.bass as bass  # noqa: F401  (engine types referenced via nc)
import concourse.tile as tile
from concourse import bacc, bass_utils, mybir

BF16 = np.dtype(ml_dtypes.bfloat16)

B, C, H, W = 16, 256, 128, 128
N_CORES = 8
GROUP = 3          # images per PSUM bank / matmul group
STRIDE = 130       # 2-col gap + 128 data cols per image in the packed layout
PAD0 = 2           # upfirdn2d pad before (both spatial dims)
TILE = 24          # images per DMA tile (must be multiple of GROUP)
QG = 2             # matmul groups (= PSUM banks / 512 cols) per chunk tile
OFF_IMGS = 12      # trailing images of each full tile W-conv'd on Vector
                   # (separable kernels only; multiple of GROUP)

_PROGRAM_CACHE: dict[object, object] = {}


def _band_matrices(kern: np.ndarray) -> np.ndarray:
    """bands[j][hi, ho] = wf[hi-ho+2, j], wf = flip(kern). Shape [4,128,128]."""
    wf = np.flip(np.asarray(kern, dtype=np.float64), (0, 1))
    bands = np.zeros((4, H, H), dtype=np.float64)
    ho = np.arange(H)
    for j in range(4):
        for i in range(4):
            d = i - PAD0            # hi - ho
            hi = ho + d
            m = (hi >= 0) & (hi < H)
            bands[j][hi[m], ho[m]] = wf[i, j]
    return np.ascontiguousarray(bands.astype(np.float32))


def _tiles(n_images: int):
    """Split n_images into DMA tiles of at most TILE images.

    The first tiles ramp up small so the first matmul can start as soon as
    a small DMA lands; any ragged remainder goes LAST so the final
    output DMA (pure tail latency) is as small as possible.
    """
    ramp = [6, 6, 12]
    out = []
    i = 0
    for r in ramp:
        if n_images - i > r:
            out.append((i, r))
            i += r
    while i < n_images:
        n = min(TILE, n_images - i)
        out.append((i, n))
        i += n
    return out


def _groups(n_images: int):
    """Split a tile's images into matmul groups of at most GROUP, avoiding a
    trailing 1-image group (rebalance 3+1 -> 2+2)."""
    out = []
    i = 0
    while i < n_images:
        n = min(GROUP, n_images - i)
        out.append((i, n))
        i += n
    if len(out) >= 2 and out[-1][1] == 1:
        i0, n0 = out[-2]
        out[-2] = (i0, 2)
        out[-1] = (i0 + 2, 2)
    return out




def build_program(n_images: int, taps=None, xt_bufs: int = 7):
    """Build + compile the per-core Bass program for n_images [128,128] images.

    DRAM layout (host-prepared, bf16):
      x: [H, n_images*STRIDE + 2]  image k's column w at STRIDE*k + 2 + w,
         cols {STRIDE*k, STRIDE*k+1} and the trailing 2 are zeros.
      y: [H, n_images*W]           image k's column w at W*k + w.

    bands[0:4] are the fused HxW band matrices (4-pass path); bands[4] is
    the H-only band (offload path).  taps, when not None, are the 4 W-conv
    scalars (flipped W factor of the separable kernel) baked as immediates;
    tiles in _offload_tiles then run: PE H-conv (1 pass) -> PSUM -> SBUF
    evac (Scalar) -> 4-tap W-conv FMA chain (Vector/GpSimd alternating).
    """
    nc = bacc.Bacc("TRN2", target_bir_lowering=False, debug=False)
    f32 = mybir.dt.float32
    bf16 = mybir.dt.bfloat16

    x_d = nc.dram_tensor("x", [H, n_images * STRIDE + 2], bf16, kind="ExternalInput")
    b_d = nc.dram_tensor("bands", [5, H, H], bf16, kind="ExternalInput")
    y_d = nc.dram_tensor("y", [H, n_images * W], bf16, kind="ExternalOutput")

    tiles = _tiles(n_images)

    with ExitStack() as ctx:
        tc = ctx.enter_context(tile.TileContext(nc))
        wpool = ctx.enter_context(tc.tile_pool(name="wpool", bufs=1))
        xpool = ctx.enter_context(tc.tile_pool(name="xpool", bufs=xt_bufs))
        opool = ctx.enter_context(tc.tile_pool(name="opool", bufs=4))
        tapool = ctx.enter_context(tc.tile_pool(name="tapool", bufs=2))
        wkpool = ctx.enter_context(tc.tile_pool(name="wkpool", bufs=6))
        ppool = ctx.enter_context(tc.tile_pool(name="ppool", bufs=4, space="PSUM"))

        wt = wpool.tile([H, 5 * H], bf16)
        nc.sync.dma_start(
            wt.rearrange("p (j b) -> p j b", b=H), b_d.rearrange("j a b -> a j b")
        )

        # Warm up the PE HAM clock gate with dummy matmuls on the weights
        # tile while the first input DMA is in flight: real matmuls then
        # start at 2.4 GHz instead of 1.2.
        warm = ppool.tile([H, 512 * QG], f32, tag="pt", name="pt")
        for _ in range(20):
            nc.tensor.matmul(
                warm[:, 0:256], wt[:, 0:H], wt[:, 0:256], start=True, stop=True
            )

        # All DMA via the two HWDGE rings (sync=SP, scalar=ACT).  GpSimd
        # (SWDGE) is kept fully idle: its Q7 descriptor-generation shares
        # SBUF ports with the Vector engine's 2-port perf modes, which the
        # W-conv chains keep busy.
        xts: dict[int, object] = {}

        def emit_in_dma(ti):
            i0, tn = tiles[ti]
            xt = xpool.tile([H, tn * STRIDE + 2], bf16, tag="xt", name="xt")
            nc.sync.dma_start(
                xt, x_d[:, i0 * STRIDE : i0 * STRIDE + tn * STRIDE + 2]
            )
            xts[ti] = xt

        for ti in range(min(6, len(tiles))):
            emit_in_dma(ti)

        def tA_view(tA, d, nseg):
            """[p, nseg, W] view of the gap-layout tile shifted by d cols."""
            span = nseg * STRIDE
            if d <= 0:
                sl = tA[:, PAD0 + d : PAD0 + d + span]
                lo = 0
            else:
                sl = tA[:, PAD0 : PAD0 + span]
                lo = d
            return sl.rearrange("p (k c) -> p k c", c=STRIDE)[:, :, lo : lo + W]

        mult = mybir.AluOpType.mult
        add = mybir.AluOpType.add

        # Output tiles are paired: two consecutive compute tiles share one
        # double-width SBUF buffer and one (larger) output DMA — halves the
        # Scalar-engine DMA issue count.  The final tile stays unpaired so
        # the very last DMA (pure tail latency) is small.
        pair_of = {}
        n_t = len(tiles)
        m = 0
        while m + 1 < n_t - 1:
            pair_of[m] = (m, m + 1)
            pair_of[m + 1] = (m, m + 1)
            m += 2

        copy_idx = 0
        ot_cur = None
        for ti, (i0, tn) in enumerate(tiles):
            if ti + 6 < len(tiles):
                emit_in_dma(ti + 6)
            xt = xts.pop(ti)

            pa, pb = pair_of.get(ti, (ti, ti))
            if ti == pa:
                pair_imgs = sum(tiles[t][1] for t in {pa, pb})
                ot_cur = opool.tile([H, pair_imgs * W], bf16, tag="ot", name="ot")
            ot0 = (i0 - tiles[pa][0]) * W  # this tile's offset into ot_cur

            # Full tiles of separable kernels split: the first (tn - seg)
            # images take the 4-pass path on PE; the trailing seg images
            # take H-conv-on-PE + W-conv-on-Vector.  Mixing both inside
            # every tile keeps PE dense (no HAM re-throttle) and feeds
            # Scalar's PSUM evacuation at a steady rate.  The last full
            # tiles stay all-PE so no Vector W-conv lands in the drain tail.
            seg = (
                OFF_IMGS
                if (taps is not None and tn == TILE and ti < n_t - 3)
                else 0
            )
            n_norm = tn - seg
            ot = ot_cur[:, ot0 : ot0 + tn * W]

            gs = _groups(n_norm)
            chunks = [gs[s : s + QG] for s in range(0, len(gs), QG)]

            for chunk in chunks:
                nq = len(chunk)
                pt = ppool.tile([H, 512 * nq], f32, tag="pt", name="pt")
                # j-outer order amortizes the 4 stationary (band) loads over
                # the whole chunk; j=2 (d=0) first for the full-width
                # has_written-clearing write.
                for idx, j in enumerate((2, 0, 1, 3)):
                    d = j - PAD0
                    for q, (goff, n) in enumerate(chunk):
                        a = PAD0
                        b = STRIDE * n + PAD0 - (PAD0 if d > 0 else 0)
                        base = goff * STRIDE
                        nc.tensor.matmul(
                            pt[:, 512 * q + a : 512 * q + b],
                            wt[:, H * j : H * (j + 1)],
                            xt[:, base + a + d : base + b + d],
                            start=(idx == 0),
                            stop=(idx == 3),
                        )

                # PSUM -> SBUF evacuation (fp32 -> bf16).  One strided copy
                # per chunk when the chunk is uniform (all groups GROUP-sized);
                # per-group copies otherwise (ragged tail).  Scalar unless no
                # offloading is active (then alternate with Vector).
                uniform = all(n == GROUP for _, n in chunk)
                if taps is not None:
                    # Scalar carries the evacuations; Vector (busy with
                    # W-conv chains) relieves it on every 8th chunk.
                    e_pick = nc.vector if copy_idx % 8 == 7 else nc.scalar
                    eng = (e_pick, e_pick)
                else:
                    eng = (nc.vector, nc.scalar)
                if uniform:
                    psrc = (
                        pt.rearrange("p (q c) -> p q c", c=512)[
                            :, :, : GROUP * STRIDE
                        ]
                        .rearrange("p q (k c) -> p q k c", c=STRIDE)[
                            :, :, :, PAD0 : PAD0 + W
                        ]
                    )
                    odst = ot[
                        :, chunk[0][0] * W : (chunk[-1][0] + GROUP) * W
                    ].rearrange("p (q k c) -> p q k c", q=nq, c=W)
                    e = eng[copy_idx % 2]
                    if e is nc.vector:
                        e.tensor_copy(odst, psrc)
                    else:
                        e.copy(odst, psrc)
                    copy_idx += 1
                else:
                    for q, (goff, n) in enumerate(chunk):
                        psrc = pt[:, 512 * q : 512 * q + STRIDE * n].rearrange(
                            "p (k c) -> p k c", c=STRIDE
                        )[:, :, PAD0 : PAD0 + W]
                        odst = ot[:, goff * W : (goff + n) * W].rearrange(
                            "p (k c) -> p k c", c=W
                        )
                        e = eng[copy_idx % 2]
                        if e is nc.vector:
                            e.tensor_copy(odst, psrc)
                        else:
                            e.copy(odst, psrc)
                        copy_idx += 1

            if seg:
                # --- offload segment: H-conv on PE, W-conv on Vector ---
                # (tensor_scalar runs 4x and tensor_add 2x even on strided
                # 3D views; scalar_tensor_tensor never leaves 1x and GpSimd
                # tensor ops are 6-45 us/op, so neither is used.)
                segc0 = n_norm * STRIDE
                span = seg * STRIDE + 2
                tA = tapool.tile([H, span], bf16, tag="ta", name="ta")
                c0 = 0
                while c0 < span:
                    cw = min(512 * QG, span - c0)
                    pt = ppool.tile([H, 512 * QG], f32, tag="pt", name="pt")
                    s = 0
                    while s < cw:
                        w_ = min(512, cw - s)
                        nc.tensor.matmul(
                            pt[:, s : s + w_],
                            wt[:, 4 * H : 5 * H],
                            xt[:, segc0 + c0 + s : segc0 + c0 + s + w_],
                            start=True,
                            stop=True,
                        )
                        s += w_
                    nc.scalar.copy(tA[:, c0 : c0 + cw], pt[:, 0:cw])
                    c0 += cw

                v = [tA_view(tA, d, seg) for d in (-2, -1, 0, 1)]
                e = nc.vector
                wk1 = wkpool.tile([H, seg * W], bf16, tag="wk", name="wk")
                wk2 = wkpool.tile([H, seg * W], bf16, tag="wk", name="wk")
                wk3 = wkpool.tile([H, seg * W], bf16, tag="wk", name="wk")
                w1v = wk1.rearrange("p (k c) -> p k c", c=W)
                w2v = wk2.rearrange("p (k c) -> p k c", c=W)
                w3v = wk3.rearrange("p (k c) -> p k c", c=W)
                otv = ot[:, n_norm * W : tn * W].rearrange("p (k c) -> p k c", c=W)
                if taps[0] == taps[3] and taps[1] == taps[2]:
                    e.tensor_add(w1v, v[0], v[3])
                    e.tensor_add(w2v, v[1], v[2])
                    e.tensor_scalar_mul(w3v, w1v, taps[0])
                    e.tensor_scalar_mul(w1v, w2v, taps[1])
                    e.tensor_add(otv, w3v, w1v)
                else:
                    e.tensor_scalar_mul(w1v, v[0], taps[0])
                    e.tensor_scalar_mul(w2v, v[1], taps[1])
                    e.tensor_add(w3v, w1v, w2v)
                    e.tensor_scalar_mul(w1v, v[2], taps[2])
                    e.tensor_add(w2v, w3v, w1v)
                    e.tensor_scalar_mul(w1v, v[3], taps[3])
                    e.tensor_add(otv, w2v, w1v)

            if ti == pb:  # pair (or singleton) complete -> one output DMA
                j0 = tiles[pa][0]
                jn = tiles[pb][0] + tiles[pb][1]
                nc.scalar.dma_start(y_d[:, j0 * W : jn * W], ot_cur)

    nc.compile()
    return nc


def _get_program(n_images: int, taps=None):
    key = (n_images, taps)
    if key not in _PROGRAM_CACHE:
        _PROGRAM_CACHE[key] = build_program(n_images, taps=taps)
    return _PROGRAM_CACHE[key]


def _separable(kern: np.ndarray):
    """Return (bands5_f32, taps) — taps None when kern is not rank-1."""
    K = np.asarray(kern, dtype=np.float64)
    bands5 = np.zeros((5, H, H), dtype=np.float32)
    bands5[0:4] = _band_matrices(kern)
    U, S, Vt = np.linalg.svd(K)
    if S[1] > 1e-6 * max(S[0], 1e-30):
        return bands5, None
    a = U[:, 0] * np.sqrt(S[0])
    b = Vt[0, :] * np.sqrt(S[0])
    af = a[::-1]  # flipped H factor
    bfl = b[::-1]  # flipped W factor -> the 4 free-dim taps
    ho = np.arange(H)
    Bh = np.zeros((H, H), dtype=np.float64)
    for i in range(4):
        hi = ho + (i - PAD0)
        m = (hi >= 0) & (hi < H)
        Bh[hi[m], ho[m]] = af[i]
    bands5[4] = Bh.astype(np.float32)
    taps = tuple(float(np.float32(v)) for v in bfl)
    return bands5, taps


def _pack_input(xc_bf16: np.ndarray) -> np.ndarray:
    """[n, H, W] bf16 -> [H, n*STRIDE + 2] bf16 gap layout."""
    n = xc_bf16.shape[0]
    arr = np.zeros((H, n * STRIDE + 2), dtype=BF16)
    v = np.lib.stride_tricks.as_strided(
        arr,
        shape=(H, n, STRIDE),
        strides=(arr.strides[0], STRIDE * arr.itemsize, arr.itemsize),
    )
    v[:, :, PAD0:] = xc_bf16.transpose(1, 0, 2)
    return arr


def kernel(x: np.ndarray, kernel: np.ndarray, _trace: bool = False):
    x = np.ascontiguousarray(x, dtype=np.float32)
    assert x.shape == (B, C, H, W), x.shape
    bands5, taps = _separable(kernel)
    bands_bf = bands5.astype(BF16)

    n_total = B * C
    n_per_core = n_total // N_CORES
    xb = x.reshape(n_total, H, W).astype(BF16)

    nc = _get_program(n_per_core, taps)
    in_maps = [
        {
            "x": _pack_input(xb[c * n_per_core : (c + 1) * n_per_core]),
            "bands": bands_bf,
        }
        for c in range(N_CORES)
    ]
    res = bass_utils.run_bass_kernel_spmd(
        nc, in_maps, core_ids=list(range(N_CORES)), trace=_trace
    )
    y = np.empty((n_total, H, W), dtype=np.float32)
    for c, r in enumerate(res.results):
        yc = np.asarray(r["y"]).reshape(H, n_per_core, W)
        y[c * n_per_core : (c + 1) * n_per_core] = yc.transpose(1, 0, 2).astype(
            np.float32
        )
    y = y.reshape(B, C, H, W)
    if _trace:
        return y, res
    return y



# revision 4
# speedup vs baseline: 1.0268x; 1.0268x over previous
"""Trainium2 Bass kernel for nn_Blur: depthwise 4x4 FIR blur (upfirdn2d pad=(2,1)).

Full inputs in, full output out. Internally shards the 4096 (b,c) images
across 8 NeuronCores (pure data parallel, no collectives).

v3: bf16 device I/O (tolerance is rel_err < 2e-2) and host pre-packing into
the SBUF gap layout (2 zero cols between images, stride 130) as in v2.

Compute per core (512 images of [H=128, W=128]) is split per-tile between
two paths, tuned so PE / Vector / Scalar all sit below the ~95us DMA floor:

 - 4-pass path (PE-heavy): the 4x4 depthwise conv factors into 4
   column-convolutions along H, each a banded matmul contracting over the
   partition (H) axis, with the W-shift (j-2) realized as a shifted
   moving-operand read of an accumulating matmul.
 - offload path (separable kernels): H-conv only on PE (1 pass), then the
   4-tap W-conv on Vector.  For taps proportional to [1,3,3,1] (the actual
   blur) the W-conv is THREE chained box-2 adds (tensor_tensor at 2x DVE
   perf mode), with the tap scale folded into the H band matrix:
     [1,3,3,1] = [1,1] * [1,1] * [1,1]  (conv)
   General symmetric/asymmetric separable taps use the 5-op FMA chain.

Unlike v2 (which kept the drain tail all-PE and the ramp all-PE), every
tile carries offload work so no engine has an idle phase: the ramp tiles
are fully offloaded (PE is HAM-cold at half clock early on), full tiles
offload ~12-15 of 24 images, and the drain overlaps PE/Vector/Scalar.

DMA: input tiles on the Sync HWDGE ring with the first x tile issued
before anything else; band matrices on the Scalar ring (idle early);
PE warm-up matmuls run on a memset tile so they need no DMA at all.
"""

import os
import sys
from contextlib import ExitStack

for _p in ("/opt/trn_rl_repo", "/root/.axon_site/_ro/trn_rl_repo"):
    if os.path.isdir(_p) and _p not in sys.path:
        sys.path.append(_p)

import ml_dtypes
import numpy as np

import concourse.bass as bass  # noqa: F401  (engine types referenced via nc)
import concourse.tile as tile
from concourse import bacc, bass_utils, mybir

BF16 = np.dtype(ml_dtypes.bfloat16)

B, C, H, W = 16, 256, 128, 128
N_CORES = 8
GROUP = 3          # images per PSUM bank / matmul group
STRIDE = 130       # 2-col gap + 128 data cols per image in the packed layout
PAD0 = 2           # upfirdn2d pad before (both spatial dims)
TILE = 24          # images per DMA tile (must be multiple of GROUP)
QG = 2             # matmul groups (= PSUM banks / 512 cols) per chunk tile

_PROGRAM_CACHE: dict[object, object] = {}


def _band_matrices(kern: np.ndarray) -> np.ndarray:
    """bands[j][hi, ho] = wf[hi-ho+2, j], wf = flip(kern). Shape [4,128,128]."""
    wf = np.flip(np.asarray(kern, dtype=np.float64), (0, 1))
    bands = np.zeros((4, H, H), dtype=np.float64)
    ho = np.arange(H)
    for j in range(4):
        for i in range(4):
            d = i - PAD0            # hi - ho
            hi = ho + d
            m = (hi >= 0) & (hi < H)
            bands[j][hi[m], ho[m]] = wf[i, j]
    return np.ascontiguousarray(bands.astype(np.float32))


def _tiles(n_images: int):
    """Split n_images into DMA tiles of at most TILE images.

    The first tiles ramp up small so the first matmul can start as soon as
    a small DMA lands; any ragged remainder goes LAST so the final
    output DMA (pure tail latency) is as small as possible.
    """
    ramp = [6, 6, 12]
    out = []
    i = 0
    for r in ramp:
        if n_images - i > r:
            out.append((i, r))
            i += r
    while i < n_images:
        n = min(TILE, n_images - i)
        out.append((i, n))
        i += n
    return out


def _off_plan(tiles, mode, off_target):
    """Per-tile offload image counts (multiples of GROUP).

    Ramp (non-full) tiles are fully offloaded (PE is HAM-cold early);
    full tiles alternate 15/12 to hit off_target; the final ragged tile
    stays all-PE (tiny, and avoids a sub-GROUP offload segment).
    """
    n_t = len(tiles)
    segs = [0] * n_t
    if mode is None:
        return segs
    total = 0
    full_idx = []
    for ti, (_, tn) in enumerate(tiles):
        if tn == TILE:
            full_idx.append(ti)
        elif ti < 3 and tn % GROUP == 0:
            segs[ti] = tn            # ramp tiles: fully offloaded
            total += tn
    want = max(0, off_target - total)
    n_full = len(full_idx)
    if n_full:
        base = min(TILE, 3 * (want // (3 * n_full)))
        extra = (want - base * n_full + 2) // 3   # tiles that get +3
        for k, ti in enumerate(full_idx):
            s = base + (3 if k < extra else 0)
            segs[ti] = min(TILE, max(0, s))
    return segs


def _groups(n_images: int):
    """Split a tile's images into matmul groups of at most GROUP, avoiding a
    trailing 1-image group (rebalance 3+1 -> 2+2)."""
    out = []
    i = 0
    while i < n_images:
        n = min(GROUP, n_images - i)
        out.append((i, n))
        i += n
    if len(out) >= 2 and out[-1][1] == 1:
        i0, n0 = out[-2]
        out[-2] = (i0, 2)
        out[-1] = (i0 + 2, 2)
    return out


def build_program(n_images: int, mode=None, taps=None, off_target=282,
                  xt_bufs: int = 7):
    """Build + compile the per-core Bass program for n_images [128,128] images.

    DRAM layout (host-prepared, bf16):
      x: [H, n_images*STRIDE + 2]  image k's column w at STRIDE*k + 2 + w,
         cols {STRIDE*k, STRIDE*k+1} and the trailing 2 are zeros.
      y: [H, n_images*W]           image k's column w at W*k + w.

    bands[0:4] are the fused HxW band matrices (4-pass path); bands[4] is
    the H-only band (offload path; for mode='box' it is pre-scaled by the
    W tap scale so the box chain needs no multiply).

    mode: None (all 4-pass) | 'box' (taps prop. to [1,3,3,1], 3-add chain)
          | 'sym' (symmetric taps, 5-op chain) | 'gen' (7-op chain).
    """
    nc = bacc.Bacc("TRN2", target_bir_lowering=False, debug=False)
    f32 = mybir.dt.float32
    bf16 = mybir.dt.bfloat16

    x_d = nc.dram_tensor("x", [H, n_images * STRIDE + 2], bf16, kind="ExternalInput")
    b_d = nc.dram_tensor("bands", [5, H, H], bf16, kind="ExternalInput")
    y_d = nc.dram_tensor("y", [H, n_images * W], bf16, kind="ExternalOutput")

    tiles = _tiles(n_images)
    segs = _off_plan(tiles, mode, off_target)

    with ExitStack() as ctx:
        tc = ctx.enter_context(tile.TileContext(nc))
        wpool = ctx.enter_context(tc.tile_pool(name="wpool", bufs=1))
        xpool = ctx.enter_context(tc.tile_pool(name="xpool", bufs=xt_bufs))
        opool = ctx.enter_context(tc.tile_pool(name="opool", bufs=4))
        tapool = ctx.enter_context(tc.tile_pool(name="tapool", bufs=2))
        wkpool = ctx.enter_context(tc.tile_pool(name="wkpool", bufs=6))
        ppool = ctx.enter_context(tc.tile_pool(name="ppool", bufs=4, space="PSUM"))

        # All DMA via the two HWDGE rings.  Input tiles ride the SP (sync)
        # ring; the band matrices + output tiles ride the ACT (scalar) ring,
        # which is idle early.  GpSimd (SWDGE) stays fully idle.
        xts: dict[int, object] = {}

        def emit_in_dma(ti):
            i0, tn = tiles[ti]
            xt = xpool.tile([H, tn * STRIDE + 2], bf16, tag="xt", name="xt")
            nc.sync.dma_start(
                xt, x_d[:, i0 * STRIDE : i0 * STRIDE + tn * STRIDE + 2]
            )
            xts[ti] = xt

        # First x tile DMA is the very first Sync-ring instruction.
        emit_in_dma(0)

        wt = wpool.tile([H, 5 * H], bf16)
        nc.scalar.dma_start(
            wt.rearrange("p (j b) -> p j b", b=H), b_d.rearrange("j a b -> a j b")
        )

        # Warm up the PE HAM clock gate with dummy matmuls on a memset tile
        # (no DMA dependency, so warm-up starts as soon as the engines come
        # out of the framework preamble): real matmuls then hit 2.4 GHz
        # within ~1-2us instead of ~5.
        wsrc = wkpool.tile([H, 256], bf16, tag="wk", name="wk")
        nc.vector.memset(wsrc, 0)
        warm = ppool.tile([H, 512 * QG], f32, tag="pt", name="pt")
        for _ in range(24):
            nc.tensor.matmul(
                warm[:, 0:256], wsrc[:, 0:H], wsrc[:, 0:256], start=True, stop=True
            )

        for ti in range(1, min(6, len(tiles))):
            emit_in_dma(ti)

        def tA_view(tA, d, nseg):
            """[p, nseg, W] view of the gap-layout tile shifted by d cols."""
            span = nseg * STRIDE
            if d <= 0:
                sl = tA[:, PAD0 + d : PAD0 + d + span]
                lo = 0
            else:
                sl = tA[:, PAD0 : PAD0 + span]
                lo = d
            return sl.rearrange("p (k c) -> p k c", c=STRIDE)[:, :, lo : lo + W]

        # Output tiles are paired: two consecutive compute tiles share one
        # double-width SBUF buffer and one (larger) output DMA — halves the
        # Scalar-engine DMA issue count.  The final tile stays unpaired so
        # the very last DMA (pure tail latency) is small.
        pair_of = {}
        n_t = len(tiles)
        m = 0
        while m + 1 < n_t - 1:
            pair_of[m] = (m, m + 1)
            pair_of[m + 1] = (m, m + 1)
            m += 2

        copy_idx = 0
        ot_cur = None
        for ti, (i0, tn) in enumerate(tiles):
            if ti + 6 < len(tiles):
                emit_in_dma(ti + 6)
            xt = xts.pop(ti)

            pa, pb = pair_of.get(ti, (ti, ti))
            if ti == pa:
                pair_imgs = sum(tiles[t][1] for t in {pa, pb})
                ot_cur = opool.tile([H, pair_imgs * W], bf16, tag="ot", name="ot")
            ot0 = (i0 - tiles[pa][0]) * W  # this tile's offset into ot_cur

            seg = segs[ti]
            n_norm = tn - seg
            ot = ot_cur[:, ot0 : ot0 + tn * W]

            gs = _groups(n_norm)
            chunks = [gs[s : s + QG] for s in range(0, len(gs), QG)]

            for chunk in chunks:
                nq = len(chunk)
                pt = ppool.tile([H, 512 * nq], f32, tag="pt", name="pt")
                # j-outer order amortizes the 4 stationary (band) loads over
                # the whole chunk; j=2 (d=0) first for the full-width
                # has_written-clearing write.
                for idx, j in enumerate((2, 0, 1, 3)):
                    d = j - PAD0
                    for q, (goff, n) in enumerate(chunk):
                        a = PAD0
                        b = STRIDE * n + PAD0 - (PAD0 if d > 0 else 0)
                        base = goff * STRIDE
                        nc.tensor.matmul(
                            pt[:, 512 * q + a : 512 * q + b],
                            wt[:, H * j : H * (j + 1)],
                            xt[:, base + a + d : base + b + d],
                            start=(idx == 0),
                            stop=(idx == 3),
                        )

                # PSUM -> SBUF evacuation (fp32 -> bf16).  One strided copy
                # per chunk when the chunk is uniform (all groups GROUP-sized);
                # per-group copies otherwise (ragged tail).  Scalar carries
                # the evacuations; Vector relieves it on every 8th chunk.
                uniform = all(n == GROUP for _, n in chunk)
                if mode is not None:
                    e_pick = nc.vector if copy_idx % 8 == 7 else nc.scalar
                    eng = (e_pick, e_pick)
                else:
                    eng = (nc.vector, nc.scalar)
                if uniform:
                    psrc = (
                        pt.rearrange("p (q c) -> p q c", c=512)[
                            :, :, : GROUP * STRIDE
                        ]
                        .rearrange("p q (k c) -> p q k c", c=STRIDE)[
                            :, :, :, PAD0 : PAD0 + W
                        ]
                    )
                    odst = ot[
                        :, chunk[0][0] * W : (chunk[-1][0] + GROUP) * W
                    ].rearrange("p (q k c) -> p q k c", q=nq, c=W)
                    e = eng[copy_idx % 2]
                    if e is nc.vector:
                        e.tensor_copy(odst, psrc)
                    else:
                        e.copy(odst, psrc)
                    copy_idx += 1
                else:
                    for q, (goff, n) in enumerate(chunk):
                        psrc = pt[:, 512 * q : 512 * q + STRIDE * n].rearrange(
                            "p (k c) -> p k c", c=STRIDE
                        )[:, :, PAD0 : PAD0 + W]
                        odst = ot[:, goff * W : (goff + n) * W].rearrange(
                            "p (k c) -> p k c", c=W
                        )
                        e = eng[copy_idx % 2]
                        if e is nc.vector:
                            e.tensor_copy(odst, psrc)
                        else:
                            e.copy(odst, psrc)
                        copy_idx += 1

            if seg:
                # --- offload segment: H-conv on PE, W-conv on Vector ---
                segc0 = n_norm * STRIDE
                span = seg * STRIDE + 2
                tA = tapool.tile([H, span], bf16, tag="ta", name="ta")
                c0 = 0
                while c0 < span:
                    cw = min(512 * QG, span - c0)
                    pt = ppool.tile([H, 512 * QG], f32, tag="pt", name="pt")
                    s = 0
                    while s < cw:
                        w_ = min(512, cw - s)
                        nc.tensor.matmul(
                            pt[:, s : s + w_],
                            wt[:, 4 * H : 5 * H],
                            xt[:, segc0 + c0 + s : segc0 + c0 + s + w_],
                            start=True,
                            stop=True,
                        )
                        s += w_
                    nc.scalar.copy(tA[:, c0 : c0 + cw], pt[:, 0:cw])
                    c0 += cw

                e = nc.vector
                otv = ot[:, n_norm * W : tn * W].rearrange("p (k c) -> p k c", c=W)
                if mode == "box":
                    # taps = s*[1,3,3,1]; s is folded into bands[4], so the
                    # W-conv is three chained box-2 adds (tensor_tensor @2x):
                    #   u1[c] = A[c] + A[c+1]          (flat, span-1 cols)
                    #   u2[c] = u1[c] + u1[c+1]        (flat, span-2 cols)
                    #   out[k,w] = u2[130k+w] + u2[130k+w+1]
                    # Gap-position garbage never reaches the read positions.
                    wk1 = wkpool.tile([H, span - 1], bf16, tag="wk", name="wk")
                    wk2 = wkpool.tile([H, span - 1], bf16, tag="wk", name="wk")
                    e.tensor_add(wk1, tA[:, 0 : span - 1], tA[:, 1:span])
                    e.tensor_add(
                        wk2[:, 0 : span - 2],
                        wk1[:, 0 : span - 2],
                        wk1[:, 1 : span - 1],
                    )
                    u2a = wk2[:, 0 : seg * STRIDE].rearrange(
                        "p (k c) -> p k c", c=STRIDE
                    )[:, :, 0:W]
                    u2b = wk2[:, 1 : seg * STRIDE + 1].rearrange(
                        "p (k c) -> p k c", c=STRIDE
                    )[:, :, 0:W]
                    e.tensor_add(otv, u2a, u2b)
                else:
                    v = [tA_view(tA, d, seg) for d in (-2, -1, 0, 1)]
                    wk1 = wkpool.tile([H, seg * W], bf16, tag="wk", name="wk")
                    wk2 = wkpool.tile([H, seg * W], bf16, tag="wk", name="wk")
                    wk3 = wkpool.tile([H, seg * W], bf16, tag="wk", name="wk")
                    w1v = wk1.rearrange("p (k c) -> p k c", c=W)
                    w2v = wk2.rearrange("p (k c) -> p k c", c=W)
                    w3v = wk3.rearrange("p (k c) -> p k c", c=W)
                    if mode == "sym":
                        e.tensor_add(w1v, v[0], v[3])
                        e.tensor_add(w2v, v[1], v[2])
                        e.tensor_scalar_mul(w3v, w1v, taps[0])
                        e.tensor_scalar_mul(w1v, w2v, taps[1])
                        e.tensor_add(otv, w3v, w1v)
                    else:
                        e.tensor_scalar_mul(w1v, v[0], taps[0])
                        e.tensor_scalar_mul(w2v, v[1], taps[1])
                        e.tensor_add(w3v, w1v, w2v)
                        e.tensor_scalar_mul(w1v, v[2], taps[2])
                        e.tensor_add(w2v, w3v, w1v)
                        e.tensor_scalar_mul(w1v, v[3], taps[3])
                        e.tensor_add(otv, w2v, w1v)

            if ti == pb:  # pair (or singleton) complete -> one output DMA
                j0 = tiles[pa][0]
                jn = tiles[pb][0] + tiles[pb][1]
                nc.scalar.dma_start(y_d[:, j0 * W : jn * W], ot_cur)

    nc.compile()
    return nc


def _get_program(n_images: int, mode=None, taps=None, off_target=282):
    key = (n_images, mode, taps, off_target)
    if key not in _PROGRAM_CACHE:
        _PROGRAM_CACHE[key] = build_program(
            n_images, mode=mode, taps=taps, off_target=off_target
        )
    return _PROGRAM_CACHE[key]


def _separable(kern: np.ndarray):
    """Return (bands5_f32, mode, taps).

    mode None: not rank-1 (all-PE 4-pass).  mode 'box': W taps proportional
    to [1,3,3,1]; the scale is folded into bands5[4] and taps is None.
    mode 'sym'/'gen': rank-1 with symmetric/general taps (5/7-op W-conv).
    """
    K = np.asarray(kern, dtype=np.float64)
    bands5 = np.zeros((5, H, H), dtype=np.float32)
    bands5[0:4] = _band_matrices(kern)
    U, S, Vt = np.linalg.svd(K)
    if S[1] > 1e-6 * max(S[0], 1e-30):
        return bands5, None, None
    a = U[:, 0] * np.sqrt(S[0])
    b = Vt[0, :] * np.sqrt(S[0])
    af = a[::-1]  # flipped H factor
    bfl = b[::-1]  # flipped W factor -> the 4 free-dim taps

    ref = np.array([1.0, 3.0, 3.0, 1.0])
    s = bfl[0]
    if abs(s) > 1e-30 and np.allclose(bfl, s * ref, rtol=1e-5, atol=0):
        mode, taps, af_eff = "box", None, af * s
    else:
        mode = "sym" if abs(bfl[0] - bfl[3]) <= 1e-7 * max(
            abs(bfl[0]), abs(bfl[3])
        ) and abs(bfl[1] - bfl[2]) <= 1e-7 * max(abs(bfl[1]), abs(bfl[2])) else "gen"
        taps, af_eff = tuple(float(np.float32(v)) for v in bfl), af

    ho = np.arange(H)
    Bh = np.zeros((H, H), dtype=np.float64)
    for i in range(4):
        hi = ho + (i - PAD0)
        m = (hi >= 0) & (hi < H)
        Bh[hi[m], ho[m]] = af_eff[i]
    bands5[4] = Bh.astype(np.float32)
    return bands5, mode, taps


def _pack_input(xc_bf16: np.ndarray) -> np.ndarray:
    """[n, H, W] bf16 -> [H, n*STRIDE + 2] bf16 gap layout."""
    n = xc_bf16.shape[0]
    arr = np.zeros((H, n * STRIDE + 2), dtype=BF16)
    v = np.lib.stride_tricks.as_strided(
        arr,
        shape=(H, n, STRIDE),
        strides=(arr.strides[0], STRIDE * arr.itemsize, arr.itemsize),
    )
    v[:, :, PAD0:] = xc_bf16.transpose(1, 0, 2)
    return arr


def kernel(x: np.ndarray, kernel: np.ndarray, _trace: bool = False):
    x = np.ascontiguousarray(x, dtype=np.float32)
    assert x.shape == (B, C, H, W), x.shape
    bands5, mode, taps = _separable(kernel)
    bands_bf = bands5.astype(BF16)

    n_total = B * C
    n_per_core = n_total // N_CORES
    xb = x.reshape(n_total, H, W).astype(BF16)

    nc = _get_program(n_per_core, mode, taps)
    in_maps = [
        {
            "x": _pack_input(xb[c * n_per_core : (c + 1) * n_per_core]),
            "bands": bands_bf,
        }
        for c in range(N_CORES)
    ]
    res = bass_utils.run_bass_kernel_spmd(
        nc, in_maps, core_ids=list(range(N_CORES)), trace=_trace
    )
    y = np.empty((n_total, H, W), dtype=np.float32)
    for c, r in enumerate(res.results):
        yc = np.asarray(r["y"]).reshape(H, n_per_core, W)
        y[c * n_per_core : (c + 1) * n_per_core] = yc.transpose(1, 0, 2).astype(
            np.float32
        )
    y = y.reshape(B, C, H, W)
    if _trace:
        return y, res
    return y
